# revision 7
# baseline (speedup 1.0000x reference)
"""Trainium2 Bass kernel for nn_Net_39230231281866 (dense_cnn).

Network: conv3x3(1->6) -> Taylor-sigmoid -> conv3x3(6->7) -> flatten
         -> fc(4032->128) -> sigmoid -> fc(128->10) -> log_softmax,
batch 8192, data-parallel over 8 NeuronCores (1024 samples/core).

Mapping (v4):
  * conv2+fc1 folded on the host into one dense GEMM W_comb [128, 4056],
    quantized fp8e4m3 (x64 scale undone by the tail sigmoid's scale) and
    run as fp8 DoubleRow matmuls (K=256/instr).
  * conv1 as 36 UNIFORM overlapping tiles of 3x7 output positions
    (window 5x9 -> K=45+bias row=46, DR split 23+23, M=126).  The conv
    BIAS is folded into the weights via a constant-1.0 window row, so
    cp = u = -(conv+b1)/2 lands in PSUM directly.  Tiles sit on PE
    row-groups {0,32,64,96} (tile_position) and stream concurrently.
  * Taylor-sigmoid T(u) = 1/(u^4+2u^3+3u^2+3u+3):
      - ~20/36 units in ONE ScalarE pass via the refit `gelu` table slot
        (scale=1.0, bias=0 -- the table maps u -> T(u) exactly).
      - ~16/36 units in ONE DVE pass via a custom op:
        m = (u(u+1)+1)^2 + (u+2)  [== q(u) exactly], then the
        bitwise-NOT reciprocal seed s = C0*bitcast(~m) (~4% rms, error
        cancels in the 4056-term GEMM; measured end-to-end ~5e-4).
  * Input host-pre-windowed into a [128, 18*1024] fp8 blob per core,
    issued as 5 chunked DMAs from the GPSIMD queue (idle early) while
    consts go on the Vector/Scalar queues -- the Sync queue's serialized
    ~0.6us-per-descriptor preamble is off the critical path.
  * One batch slice (512 samples) at a time: cp tiles [128,1024] hold a
    tile-pair x one slice (2 PSUM banks, 3 buffers); GEMM software-
    pipelined _GEMM_LAG units behind.
  * Tail: h = 0.5*tanh(z/2)+0.5 with the affine folded into fc2 weights
    and bias (tanh lives in the already-loaded gelu set); log_softmax
    with Exp/Ln pinned to one table (single swap); output as ONE
    [128,80] DMA, un-permuted on the host.
"""

import os
import numpy as np
import ml_dtypes

_B = 8192
_NCORES = 8
_PC = _B // _NCORES          # 1024 samples per core
_SLICE = 512
_NSL = _PC // _SLICE         # 2 batch slices

# uniform conv1 tiling with overlap: rows 0,3,..,21,23 cols 0,7,14,19
_OYS = [0, 3, 6, 9, 12, 15, 18, 21, 23]
_OXS = [0, 7, 14, 19]
_NOY, _NOX = 3, 7            # output positions per tile
_KY, _KX = 5, 9              # input window per tile
_K = _KY * _KX               # 45 (+1 bias row = 46)
_M = _NOY * _NOX * 6         # 126
_NT = len(_OYS) * len(_OXS)  # 36 tiles
_NPAIR = _NT // 2            # 18 row-group pairs

_GEMM_LAG = 5                # software pipeline depth (units)
_N_WARM = 6                  # HAM warm-up matmuls before the first conv
_WCSCALE = 64.0              # fp8 scale for W_comb; undone in the sigmoid
_SEED_C0 = -0.23549792       # Chebyshev recip-seed scale

_N_DVE = 16                  # units (of 36) on the DVE seed path


def _dve_units():
    """Evenly-spread unit ids (uid = sl*18 + p) for the DVE path."""
    flags = [False] * (_NPAIR * _NSL)
    n = len(flags)
    for i in range(_N_DVE):
        flags[(i * n) // _N_DVE] = True
    return flags


LAST_RESULTS = None


def _tiles():
    return [(oy, ox) for oy in _OYS for ox in _OXS]


def _host_prep(x, w1, b1, w2, b2, fw1, fb1, fw2, fb2):
    x = np.asarray(x, np.float32)
    w1 = np.asarray(w1, np.float32); b1 = np.asarray(b1, np.float32)
    w2 = np.asarray(w2, np.float32); b2 = np.asarray(b2, np.float32)
    fw1 = np.asarray(fw1, np.float32); fb1 = np.asarray(fb1, np.float32)
    fw2 = np.asarray(fw2, np.float32); fb2 = np.asarray(fb2, np.float32)

    tiles = _tiles()

    # shared banded conv1 weights [46, 126], scaled by -1/2 so that
    # cp = u = -(conv+b1)/2; row 45 is the bias row (window value 1.0)
    w1pack = np.zeros((_K + 1, _M), np.float32)
    for dy in range(_NOY):
        for dx in range(_NOX):
            for oc in range(6):
                m = (dy * _NOX + dx) * 6 + oc
                for ky in range(3):
                    for kx in range(3):
                        k = (dy + ky) * _KX + (dx + kx)
                        w1pack[k, m] = -0.5 * w1[oc, 0, ky, kx]
                w1pack[_K, m] = -0.5 * b1[oc]

    # fold conv2 + fc1 -> W_comb [128, 6*26*26] (x1.5: s = 1.5*T), b_comb
    fw1r = fw1.reshape(128, 7, 24, 24)
    Wc = np.zeros((128, 6, 26, 26), np.float32)
    for dy in range(3):
        for dx in range(3):
            Wc[:, :, dy:dy + 24, dx:dx + 24] += np.einsum(
                "joyx,oi->jiyx", fw1r, w2[:, :, dy, dx], optimize=True)
    b_comb = fb1 + np.einsum("joyx,o->j", fw1r, b2)
    Wc_flat = (1.5 * Wc.reshape(128, 6 * 26 * 26)).astype(np.float32)

    # W_comb columns packed per tile [128, 36*128]; overlapped (duplicate)
    # output positions are owned by the first tile that produces them.
    owned = np.zeros((26, 26), bool)
    wcpack = np.zeros((128, 128 * _NT), np.float32)
    for t_i, (oy0, ox0) in enumerate(tiles):
        for dy in range(_NOY):
            for dx in range(_NOX):
                y, xq = oy0 + dy, ox0 + dx
                if owned[y, xq]:
                    continue
                owned[y, xq] = True
                for oc in range(6):
                    m = (dy * _NOX + dx) * 6 + oc
                    pos = (oc * 26 + y) * 26 + xq
                    wcpack[m, 128 * t_i:128 * t_i + 128] = Wc_flat[:, pos]
    assert owned.all()

    # DoubleRow stationary layout: per pair p -> [128, 2*128] fp8,
    # cols [j*128 + m] = wcpack block of tile 2p+j, scaled by _WCSCALE
    f8 = ml_dtypes.float8_e4m3fn
    wcpk8 = np.zeros((128, 256 * _NPAIR), f8)
    for p in range(_NPAIR):
        for j in range(2):
            t_i = 2 * p + j
            wcpk8[:, 256 * p + 128 * j:256 * p + 128 * j + 128] = (
                _WCSCALE * wcpack[:, 128 * t_i:128 * t_i + 128]).astype(f8)

    # f32 const blob [128, 41]: col0 = tanh bias (0.5*b_comb), 1:41 = fb2r
    cst32 = np.zeros((128, 41), np.float32)
    cst32[:, 0] = 0.5 * b_comb
    cst32[:, 1:41] = np.tile((fb2 + 0.5 * fw2.sum(axis=1)).reshape(1, 10),
                             (128, 4)).astype(np.float32)

    consts = dict(
        wcpack=wcpk8,
        cst32=cst32,
        # fc2 folded for h = 0.5*tanh(z/2) + 0.5
        fw2t=np.ascontiguousarray(0.5 * fw2.T).astype(np.float16),  # [128,10]
    )
    # w1pack in DoubleRow SBUF layout [128, 256] fp8: at each row group
    # 32g, partition k (k<23), col h*128+m = w1pack[h*23+k, m]
    _KH = 23
    w1dr = np.zeros((_KH, 256), np.float32)     # [k, (h, m-padded-128)]
    for h in range(2):
        for k in range(_KH):
            r = h * _KH + k
            if r <= _K:
                w1dr[k, h * 128:h * 128 + _M] = w1pack[r, :]
    w1sb = np.zeros((128, 256), np.float32)
    for g in range(4):
        w1sb[32 * g:32 * g + _KH, :] = w1dr
    consts["w1pack"] = w1sb.astype(ml_dtypes.float8_e4m3fn)

    # window pixel indices per tile (46 rows, last is the bias row)
    x_pm = x.reshape(_B, 784).T.astype(np.float32)                      # [784, B]
    row_idx = []
    for (oy0, ox0) in tiles:
        rows = ((np.arange(_KY)[:, None] + oy0) * 28 +
                (np.arange(_KX)[None, :] + ox0)).reshape(-1)
        row_idx.append(rows)
    return x_pm, row_idx, consts, tiles


_PWP_BASE = ('/nix/store/z022hj2nvbm3nwdizlisq4ylc0y7rd6q-python3-3.13.14-env/'
             'lib/python3.13/site-packages/neuronxcc/pwp/pwp_bin_trainium')


def _taylor_T(x):
    x = np.asarray(x, np.float64)
    return 1.0 / ((((x + 2.0) * x + 3.0) * x + 3.0) * x + 3.0)


def _make_act_root():
    """Build a private act-table root where the `gelu` slot of
    gelu_and_others evaluates T(u) = 1/(u^4+2u^3+3u^2+3u+3) — the whole
    Taylor-sigmoid denominator-reciprocal — as one ScalarE pass."""
    import json as _json
    import shutil
    import tempfile
    root = tempfile.mkdtemp(prefix="ant_act_root_")
    for fn in os.listdir(_PWP_BASE):
        shutil.copy(os.path.join(_PWP_BASE, fn), root)
    setp = os.path.join(root, 'gelu_and_others.json')
    meta = _json.load(open(setp))
    bktp = os.path.join(root, 'gelu_and_others_bkt.bin')
    bkt = np.fromfile(bktp, dtype=np.float32).reshape(-1, 8).copy()

    fx = meta['func_exp_to_bkt_start_idx']['gelu']
    ends = {0: 443, 1: 504}

    def seg_info(e, side):
        start = fx[str(e)][side]
        n = None
        for e2 in range(e + 1, 9):
            if str(e2) in fx and len(fx[str(e2)]) > side:
                n = fx[str(e2)][side] - start
                break
        if n is None:
            n = ends[side] - start
        return start, n

    def fit(idx, a, b):
        x0 = bkt[idx, 4]
        xs = np.linspace(a, b, 33)
        t = xs - x0
        co = np.polyfit(t, _taylor_T(xs), 3)   # [c3, c2, c1, c0]
        bkt[idx, 0:4] = co[::-1]

    for side, sgn in ((0, -1.0), (1, 1.0)):
        emax = 3 if side == 0 else 2
        for e in range(-7, emax + 1):
            start, n = seg_info(e, side)
            w = (2.0 ** e) / n
            for k in range(n):
                lo = 2.0 ** e + k * w
                a, b = sgn * lo, sgn * (lo + w)
                fit(start + k, min(a, b), max(a, b))
    # small-signal controls (|x| < 2^-7): 504 pos, 505 neg
    bkt[504, 4] = 0.0
    fit(504, 0.0, 2.0 ** -7)
    bkt[505, 4] = 0.0
    fit(505, -(2.0 ** -7), 0.0)
    # large-signal controls: T ~ 0 out there (unreachable for real data)
    bkt[506, 0:5] = 0.0
    bkt[507, 0:5] = 0.0
    bkt.tofile(bktp)

    for pm in meta['profile_meta_data']:
        if pm['func_name'] == 'gelu_4p':
            pm['fzero_result'] = int(
                np.float32(1.0 / 3.0).view(np.uint32))
            pm['fpinf_result'] = 0
            pm['fninf_result'] = 0
    _json.dump(meta, open(setp, 'w'))
    return os.path.join(root, 'act_info.json')


def _register_taylor_seed():
    """One-pass DVE Taylor-sigmoid: s = C0 * bitcast(~bitcast(q(u)))
    with q(u) = (u(u+1)+1)^2 + (u+2)  (exact identity for the quartic)."""
    import concourse.dve_ops as dve_ops
    if "TAYLOR_SEED_ANT" in dve_ops._SUB_OPCODE_FOR_NAME:
        return next(o for o in dve_ops.OPS if o.name == "TAYLOR_SEED_ANT")
    from concourse.dve_spec import Spec, Src0, C0, One, Bin, AluOp, lower
    from concourse.dve_spec import _has_src1
    from concourse.dve_uop import DveOpSpec

    a = Src0 + One
    g = a * Src0 + One
    m = g * g + (a + One)
    body = Bin(AluOp.BITWISE_NOT, m, m) * C0

    def _ref(in0, in1, s0, s1, imm2):
        v = in0.astype(np.float32)
        mm = np.float32(1) + (v + np.float32(1)) * v
        mm = mm * mm + (v + np.float32(2))
        nxr = (~mm.astype(np.float32).view(np.int32)).view(np.float32)
        return np.float32(s0) * nxr

    spec = Spec(body=body, reference=_ref)
    row = max(dve_ops._SUB_OPCODE_FOR_NAME.values()) + 1
    assert row < 0x20
    shas = {}
    for ver in ("v3", "v4"):
        try:
            compiled = DveOpSpec(name="TAYLOR_SEED_ANT", opcode=row,
                                 uops=lower(spec, ver=ver),
                                 rd1_en=_has_src1(spec))
            shas[ver] = compiled.sha(ver)
        except Exception:
            pass
    op = dve_ops.DveOp("TAYLOR_SEED_ANT", spec, subdim=False, uops_sha=shas)
    dve_ops.OPS.append(op)
    dve_ops.CUSTOM_DVE_SPECS[op.name] = op.spec
    dve_ops._SUB_OPCODE_FOR_NAME[op.name] = row
    return op


def _pin_exp_ln_table():
    """Make Exp and Ln resolve only to natural_log_exp_and_others so the
    log_softmax tail costs one table load instead of alternating sets."""
    import concourse.bacc as bacc
    import concourse.mybir as mybir
    if getattr(bacc, "_ant_expln_pinned", False):
        return
    orig = bacc.get_activation_tables
    AF = mybir.ActivationFunctionType

    def patched(arch):
        tabs = {k: set(v) for k, v in orig(arch).items()}
        for name, fns in tabs.items():
            if name != "natural_log_exp_and_others":
                fns.discard(AF.Exp)
                fns.discard(AF.Ln)
            if name != "gelu_and_others":
                fns.discard(AF.Tanh)
        return tabs

    bacc.get_activation_tables = patched
    bacc._ant_expln_pinned = True


def _build_program():
    import concourse.bacc as bacc
    import concourse.mybir as mybir
    from concourse.tile import TileContext
    from concourse.alu_op_type import AluOpType

    f32 = mybir.dt.float32
    f16 = mybir.dt.float16
    AF = mybir.ActivationFunctionType
    taylor_seed = _register_taylor_seed()
    _pin_exp_ln_table()

    f8 = mybir.dt.float8e4
    os.environ["BASS_ACT_ROOT_JSON_PATH"] = _make_act_root()
    nc = bacc.Bacc()
    xwin = nc.declare_dram_parameter("xwin", [128, _NPAIR * _PC], f8,
                                     isOutput=False)
    wcpack_d = nc.declare_dram_parameter("wcpack", [128, 256 * _NPAIR], f8,
                                         isOutput=False)
    w1pack_d = nc.declare_dram_parameter("w1pack", [128, 256], f8,
                                         isOutput=False)
    cst32_d = nc.declare_dram_parameter("cst32", [128, 41], f32,
                                        isOutput=False)
    fw2t_d = nc.declare_dram_parameter("fw2t", [128, 10], f16, isOutput=False)
    out_d = nc.declare_dram_parameter("out", [128, 10 * 8], f32,
                                      isOutput=True)

    # xwin DMA chunks: (slice, quad-range); tiny first chunk so the
    # first conv starts as soon as the framework preamble ends
    _CHUNKS = [(0, 0, 1), (0, 1, 3), (0, 3, 6), (0, 6, 9), (1, 0, 9)]
    dve_flags = _dve_units()

    with TileContext(nc) as tc:
        with (
            tc.tile_pool(name="const", bufs=1) as cpool,
            tc.tile_pool(name="xw", bufs=1) as xpool,
            tc.tile_pool(name="work", bufs=7) as wpool,
            tc.tile_pool(name="cps", bufs=3, space="PSUM") as cps,
            tc.tile_pool(name="zps", bufs=2, space="PSUM") as zps,
        ):
            # DMA issue split over the two HWDGE queues: Scalar gets the
            # launch-critical tiles (xw0/w1pack), Sync streams the rest
            xts = []
            for c, (csl, q0, q1) in enumerate(_CHUNKS):
                t = xpool.tile([128, (q1 - q0) * 1024], f8, tag=f"xw{c}",
                               name=f"xw{c}", bufs=1)
                xts.append(t)
            nc.scalar.dma_start(out=xts[0], in_=xwin[:, 0:1024])
            w1pack_sb = cpool.tile([128, 256], f8, name="w1pack_sb")
            nc.scalar.dma_start(out=w1pack_sb, in_=w1pack_d[:])
            cst32_sb = cpool.tile([128, 41], f32, name="cst32_sb")
            nc.scalar.dma_start(out=cst32_sb, in_=cst32_d[:])
            fw2t_sb = cpool.tile([128, 10], f16, name="fw2t_sb")
            nc.scalar.dma_start(out=fw2t_sb, in_=fw2t_d[:])
            for c in range(1, 3):
                csl, q0, q1 = _CHUNKS[c]
                nc.sync.dma_start(
                    out=xts[c], in_=xwin[:, 9216 * csl + 1024 * q0:
                                         9216 * csl + 1024 * q1])
            wcpack_sb = cpool.tile([128, 256 * _NPAIR], f8, name="wcpack_sb")
            nc.sync.dma_start(out=wcpack_sb, in_=wcpack_d[:])
            for c in range(3, len(_CHUNKS)):
                csl, q0, q1 = _CHUNKS[c]
                nc.sync.dma_start(
                    out=xts[c], in_=xwin[:, 9216 * csl + 1024 * q0:
                                         9216 * csl + 1024 * q1])
            bcomb_sb = cst32_sb[:, 0:1]
            fb2r_sb = cst32_sb[:, 1:41]

            def xw_ap(t, sl):
                """DoubleRow rhs [23, 2, 512] for tile t, slice sl."""
                quad, g = t // 4, t % 4
                for c, (csl, q0, q1) in enumerate(_CHUNKS):
                    if csl == sl and q0 <= quad < q1:
                        break
                base = (quad - q0) * 1024
                return xts[c][32 * g:32 * g + 23,
                              base:base + 1024].rearrange(
                                  "p (h n) -> p h n", n=_SLICE)

            zs = [zps.tile([128, _SLICE], f32, tag=f"z{sl}", name=f"z{sl}",
                           bufs=1)
                  for sl in range(_NSL)]

            # single-sync-wait rule: pre-observe PE-read const queues with
            # dummy 1-col matmuls (into z0; cleared by the first start=True
            # GEMM); ACT/DVE-read consts with dummy touches.
            nc.tensor.matmul(zs[0][0:126, 0:1], w1pack_sb[0:23, 0:126],
                             w1pack_sb[0:23, 0:1], start=True, stop=True)

            # HAM warm-up: the PE clock sits at 1.2 GHz until it has been
            # busy for a full free-running 4096-cycle window (~3.4us).  The
            # dependency-paced main loop never streams gap-free that long,
            # so it would run cold end-to-end.  Burn ~6 back-to-back N=512
            # matmuls (gated only on xw0+w1pack, overlapping the remaining
            # DMA wait) so the first real convs tip the window over.
            wv_warm = w1pack_sb.rearrange("p (j m) -> p j m", j=2)
            for i in range(_N_WARM):
                nc.tensor.matmul(
                    zs[i % 2], wv_warm,
                    xts[0][:, 0:1024].rearrange("p (j n) -> p j n", j=2),
                    start=True, stop=True,
                    perf_mode=mybir.MatmulPerfMode.DoubleRow)

            def emit_late_dummies():
                # pre-observe the GEMM/fc2 const queues late so they do not
                # gate the first conv on the big wcpack DMA
                nc.tensor.matmul(zs[0][0:128, 0:1], wcpack_sb[0:128, 0:128],
                                 wcpack_sb[0:128, 0:1], start=True, stop=True)
                nc.tensor.matmul(zs[0][0:10, 0:1], fw2t_sb[0:128, 0:10],
                                 fw2t_sb[0:128, 0:1], start=True, stop=True)
            dvescr = wpool.tile([128, 40], f32, tag="dvescr", name="dvescr",
                                bufs=1)
            nc.vector.tensor_copy(out=dvescr[:, 0:40], in_=fb2r_sb)
            actscr = wpool.tile([128, 1], f32, tag="actscr", name="actscr",
                                bufs=1)
            nc.scalar.copy(out=actscr[:], in_=bcomb_sb)

            ss = {}

            def conv_mm(t, cp, half, sl):
                g = t % 4
                wv = w1pack_sb[32 * g:32 * g + 23, :].rearrange(
                    "p (h m) -> p h m", h=2)   # [23, 2, 128]
                nc.tensor.matmul(
                    cp[0:128, half * _SLICE:(half + 1) * _SLICE],
                    wv, xw_ap(t, sl), start=True, stop=True,
                    perf_mode=mybir.MatmulPerfMode.DoubleRow,
                    tile_position=(32 * g, 0))

            def emit_pair(p, sl):
                # cp holds tiles (2p, 2p+1) for batch slice sl
                cp = cps.tile([128, _PC], f32, tag="cp", name=f"cp{p}_{sl}")
                conv_mm(2 * p, cp, 0, sl)
                conv_mm(2 * p + 1, cp, 1, sl)
                s = wpool.tile([128, _PC], f8, tag="s", name=f"s{p}_{sl}")
                if not dve_flags[sl * _NPAIR + p]:
                    # one-pass Taylor-sigmoid via the hijacked gelu table:
                    # s = T(cp) = T(u)
                    nc.scalar.activation(s, cp, AF.Gelu, bias=0.0, scale=1.0)
                else:
                    # one-pass DVE: quartic + bitwise-NOT reciprocal seed
                    nc.vector._custom_dve(
                        taylor_seed, out=s, in0=cp, s0=_SEED_C0)
                ss[(p, sl)] = s

            def emit_gemm(p, sl):
                s = ss.pop((p, sl))
                # s layout [128, (j 2) (n 512)] -> DoubleRow rhs
                sv = s.rearrange("p (j n) -> p j n", j=2)
                wv = wcpack_sb[:, 256 * p:256 * (p + 1)].rearrange(
                    "p (j m) -> p j m", j=2)
                nc.tensor.matmul(
                    zs[sl], wv, sv,
                    start=(p == 0), stop=(p == _NPAIR - 1),
                    perf_mode=mybir.MatmulPerfMode.DoubleRow)

            # mid-stream per-slice tail (gelu table only): tanh -> fc2 ->
            # +fb2; the slice-0 instance overlaps slice-1's conv/GEMM work.
            ng = _SLICE // 128
            lgall = wpool.tile([128, 10 * ng * _NSL], f32, tag="lg",
                               name="lgall", bufs=1)

            def emit_mid_tail(sl):
                h = wpool.tile([128, _SLICE], f16, tag="h", name=f"h{sl}")
                nc.scalar.activation(h, zs[sl], AF.Tanh, bias=bcomb_sb,
                                     scale=0.5 / _WCSCALE)
                fp = cps.tile([128, 10 * ng], f32, tag="cp", name=f"fp{sl}")
                for g in range(ng):
                    nc.tensor.matmul(fp[:, g * 10:(g + 1) * 10],
                                     h[:, g * 128:(g + 1) * 128],
                                     fw2t_sb[:], start=True, stop=True)
                nc.vector.tensor_tensor(
                    out=lgall[:, sl * 40:(sl + 1) * 40], in0=fp,
                    in1=fb2r_sb, op=AluOpType.add)

            for sl in range(_NSL):
                for p in range(_NPAIR):
                    emit_pair(p, sl)
                    if sl == 0 and p == 1:
                        emit_late_dummies()
                    if p >= _GEMM_LAG:
                        emit_gemm(p - _GEMM_LAG, sl)
                for p in range(_NPAIR - _GEMM_LAG, _NPAIR):
                    emit_gemm(p, sl)
                emit_mid_tail(sl)

            # ---- end tail: ONE table swap, then exp / reduce / ln / sub
            # over both slices at once (no max-sub: |logits| < 12).
            e = wpool.tile([128, 10 * ng * _NSL], f32, tag="e", name="e")
            nc.scalar.activation(e, lgall, AF.Exp)
            ssum = wpool.tile([128, ng * _NSL], f32, tag="ss", name="ss")
            nc.vector.tensor_reduce(
                ssum, e.rearrange("p (g k) -> p g k", k=10),
                axis=mybir.AxisListType.X, op=AluOpType.add)
            lns = wpool.tile([128, ng * _NSL], f32, tag="ls", name="ls")
            nc.scalar.activation(lns, ssum, AF.Ln)
            ot = wpool.tile([128, 10 * ng * _NSL], f32, tag="ot", name="ot")
            for g in range(ng * _NSL):
                eng = nc.vector if g % 2 == 0 else nc.gpsimd
                eng.tensor_scalar(
                    out=ot[:, g * 10:(g + 1) * 10],
                    in0=lgall[:, g * 10:(g + 1) * 10],
                    scalar1=lns[:, g:g + 1], scalar2=None,
                    op0=AluOpType.subtract)
            nc.sync.dma_start(out=out_d[:], in_=ot)
    nc.compile()
    return nc


_PROGRAM_CACHE = {}


def kernel(x, w1, b1, w2, b2, fw1, fb1, fw2, fb2):
    global LAST_RESULTS
    x_pm, row_idx, consts, tiles = _host_prep(
        x, w1, b1, w2, b2, fw1, fb1, fw2, fb2)

    if "nc" not in _PROGRAM_CACHE:
        _PROGRAM_CACHE["nc"] = _build_program()
    nc = _PROGRAM_CACHE["nc"]

    shared = {k: consts[k] for k in
              ("wcpack", "w1pack", "cst32", "fw2t")}
    in_maps = []
    for c in range(_NCORES):
        m = dict(shared)
        xc = x_pm[:, c * _PC:(c + 1) * _PC]                 # [784, 1024]
        # layout per tile t (quad=t//4, g=t%4): partitions 32g..32g+22,
        # cols 9216*sl + 1024*quad + 512*h + b  =  win[h*23+k, sample]
        blob = np.zeros((128, _NPAIR * _PC), ml_dtypes.float8_e4m3fn)
        for t in range(_NT):
            quad, g = t // 4, t % 4
            rows = row_idx[t]                               # 45 pixel indices
            wp = np.zeros((46, _PC), np.float32)
            wp[:45] = xc[rows, :]
            wp[45] = 1.0                                    # bias row
            for sl in range(_NSL):
                for h in range(2):
                    blob[32 * g:32 * g + 23,
                         9216 * sl + 1024 * quad + 512 * h:
                         9216 * sl + 1024 * quad + 512 * (h + 1)] = \
                        wp[h * 23:(h + 1) * 23,
                           sl * _SLICE:(sl + 1) * _SLICE]
        m["xwin"] = blob
        in_maps.append(m)

    from concourse.bass_utils import run_bass_kernel_spmd
    trace = bool(int(os.environ.get("BASS_KERNEL_TRACE", "0")))
    res = run_bass_kernel_spmd(nc, in_maps, core_ids=list(range(_NCORES)),
                               trace=trace)
    LAST_RESULTS = res
    # out[p, sl*40 + g*10 + k] -> sample sl*512 + g*128 + p
    outs = []
    for r in res.results:
        o = np.asarray(r["out"]).reshape(128, _NSL, 4, 10)
        outs.append(o.transpose(1, 2, 0, 3).reshape(_PC, 10))
    return np.concatenate(outs, axis=0)
